# revision 28
# baseline (speedup 1.0000x reference)
"""BinaryContrastiveLoss Trainium2 kernel (v4, 224-231us; v3 baseline 264.9us).

Contract: kernel(**inputs) takes the FULL unsharded inputs
  features:       [8, 4096, 128] float32
  positive_index: [8, 4096, 16]  int64
  negative_index: [8, 4096, 32]  int64
and returns the scalar loss (np.float32), matching reference().

Sharding: data-parallel over the batch dim B=8 -> 8 NeuronCores, one
batch element per core.  All gathers are local to a batch element.
Each core computes S_b = sum_n sum_p log1p(exp(pos_dot - ln(denom_n)));
host combines: loss = mean_b( -S_b / (P*N) ).

Device algorithm per core (N=4096 tokens, D=128, P=16, Q=32):
  phase 1: load features in one DMA, batched L2-normalize on DVE
           (sqrt on ACT first so the exp/ln table set loads once),
           bf16 normalized copy -> DRAM gather table; xbar
           dma_start_transpose -> FTbf [d, N] bf16; DVE cast -> FT8 fp8.
  phase 2 per 128-token tile:
    - numerator: 2x 1024-idx dma_gather (bf16 rows, slot-sorted on
      host for HBM locality), DVE multiply + halving tree.
    - denominator: fp8 Gram in two 2048-col f32 PSUM chunks (8x 512-col
      matmuls); ln(count of all 48 indices, fp8, -16 at zero) added via
      identity-lhsT matmuls accumulating into PSUM (PE, both chunks —
      keeps DVE off the slow 1x PSUM path); ACT exp+accum per chunk.
  phase 3: batched epilogue: one TT shift, one exp, one log1p+accum,
           PE ones-matmul partition reduction, scalar out.
"""

import sys

if "/opt/trn_rl_repo" not in sys.path:
    sys.path.insert(0, "/opt/trn_rl_repo")

import numpy as np

B, N, D, P, Q = 8, 4096, 128, 16, 32
TILE = 128
NT = N // TILE
CHUNK = 2048
NCH = N // CHUNK          # denominator chunks per tile
PIECE = 512               # psum bank width (f32)
KC = 8                    # k's per dma_gather call (1024-idx ring limit)
NCALL = P // KC           # gather calls per tile

_CACHE = {}


def build_program():
    if "nc" in _CACHE:
        return _CACHE["nc"]

    from concourse import bacc, bass, mybir, tile

    f32 = mybir.dt.float32
    bf16 = mybir.dt.bfloat16
    fp8 = mybir.dt.float8e4
    i16 = mybir.dt.int16
    AF = mybir.ActivationFunctionType
    ALU = mybir.AluOpType

    nc = bacc.Bacc(None, target_bir_lowering=False, num_swdge_queues=4)
    feats = nc.dram_tensor("features", [N, D], f32, kind="ExternalInput")
    # positive gather indices: per (tile, call) 1024 idx, k-major flat
    # order, wrapped 16-way and replicated across the 8 partition groups
    idxw = nc.dram_tensor(
        "idxw", [NT, NCALL, 128, KC * TILE // 16], i16, kind="ExternalInput"
    )
    # ln(counts of all 48 indices) per token row, fp8, -16 at zero counts
    lnc8 = nc.dram_tensor(
        "lnc8", [NT, NCH, 128, CHUNK], fp8, kind="ExternalInput"
    )
    i8 = nc.dram_tensor("i8", [128, 128], fp8, kind="ExternalInput")
    out = nc.dram_tensor("out", [1, 1], f32, kind="ExternalOutput")
    table = nc.dram_tensor("table", [N, D], bf16)       # gather source

    with tile.TileContext(nc) as tc:
        with (
            tc.tile_pool(name="const", bufs=1) as cpool,
            tc.tile_pool(name="work", bufs=3) as work,
            tc.tile_pool(name="lnc", bufs=10) as lpool,
            tc.tile_pool(name="gather", bufs=10) as gpool,
            tc.tile_pool(name="psum", bufs=2, space="PSUM") as psum,
        ):
            idn = cpool.tile([128, 128], fp8)
            idx_all = cpool.tile([128, NT, NCALL, KC * TILE // 16], i16)
            R_all = cpool.tile([TILE, NT, P], f32)
            den2 = cpool.tile([TILE, NT, NCH], f32)
            ones = cpool.tile([TILE, 1], f32)
            nc.vector.memset(ones[:], 1.0)

            # ---- phase 1: load + normalize + table + transposes ----
            ft_all = cpool.tile([TILE, NT, D], f32)
            nc.sync.dma_start(
                out=ft_all[:], in_=feats[:].rearrange("(t p) d -> p t d", p=TILE)
            )
            nc.sync.dma_start(out=idn[:], in_=i8[:])
            nc.sync.dma_start(
                out=idx_all[:], in_=idxw[:].rearrange("t c p s -> p t c s")
            )
            sqc = cpool.tile([TILE, NT, D], f32)
            ss_all = cpool.tile([TILE, NT], f32)
            rs_all = cpool.tile([TILE, NT], f32)
            ri_all = cpool.tile([TILE, NT], f32)
            nc.vector.tensor_tensor(
                out=sqc[:], in0=ft_all[:], in1=ft_all[:], op=ALU.mult,
            )
            nc.vector.tensor_reduce(
                out=ss_all[:], in_=sqc[:], axis=mybir.AxisListType.X, op=ALU.add,
            )
            nc.vector.reciprocal(rs_all[:], ss_all[:])
            nc.scalar.sqrt(ri_all[:], rs_all[:])
            # batched normalize straight to bf16 (broadcast middle scale)
            fnbf = cpool.tile([TILE, NT, D], bf16)
            nc.vector.tensor_tensor(
                out=fnbf[:],
                in0=ft_all[:],
                in1=ri_all[:].unsqueeze(2).broadcast_to([TILE, NT, D]),
                op=ALU.mult,
            )
            nc.sync.dma_start(
                out=table[:].rearrange("(t p) d -> p t d", p=TILE), in_=fnbf[:]
            )
            # transposed copies: FTbf [d, token] bf16 via xbar, FT8 fp8 cast
            FTbf = cpool.tile([128, N], bf16)
            nc.sync.dma_start_transpose(out=FTbf[:], in_=table[:])
            FT8 = cpool.tile([128, N], fp8)
            nc.vector.tensor_copy(FT8[:], FTbf[:])

            # ---- phase 2: per-tile numerator + denominator ----
            for t in range(NT):
                tok = slice(t * TILE, (t + 1) * TILE)

                # numerator: 1024-idx gather calls + bf16 dot tree on DVE
                g = gpool.tile([TILE, P, D], bf16, tag="g")
                for gc in range(NCALL):
                    nc.gpsimd.dma_gather(
                        out_ap=g[:, gc * KC : (gc + 1) * KC, :],
                        in_ap=table[:],
                        idxs_ap=idx_all[:, t, gc, :],
                        num_idxs=KC * TILE,
                        num_idxs_reg=KC * TILE,
                        elem_size=D,
                        queue_num=(t * NCALL + gc) % 4,
                        single_packet=False,
                    )
                prod = work.tile([TILE, P, D], bf16, tag="prod", bufs=2)
                h1 = work.tile([TILE, P, D // 2], bf16, tag="h1", bufs=2)
                h2 = work.tile([TILE, P, D // 4], bf16, tag="h2", bufs=2)
                nc.vector.tensor_tensor(
                    out=prod[:],
                    in0=g[:],
                    in1=fnbf[:, t, :].unsqueeze(1).broadcast_to([TILE, P, D]),
                    op=ALU.mult,
                )
                nc.vector.tensor_tensor(
                    out=h1[:], in0=prod[:, :, 0 : D // 2],
                    in1=prod[:, :, D // 2 : D], op=ALU.add,
                )
                nc.vector.tensor_tensor(
                    out=h2[:], in0=h1[:, :, 0 : D // 4],
                    in1=h1[:, :, D // 4 : D // 2], op=ALU.add,
                )
                nc.vector.tensor_reduce(
                    out=R_all[:, t, :], in_=h2[:],
                    axis=mybir.AxisListType.X, op=ALU.add,
                )

                # denominator: Gram + lnc (both on PE), exp+accum on ACT
                ldt = lpool.tile([128, NCH, CHUNK], fp8, tag="ldt")
                nc.sync.dma_start(out=ldt[:], in_=lnc8[t].rearrange("c p n -> p c n"))
                gr0 = psum.tile([TILE, CHUNK], f32, tag="gr0", bufs=1)
                gr1 = psum.tile([TILE, CHUNK], f32, tag="gr1", bufs=1)
                ej = work.tile([TILE, CHUNK], bf16, tag="ej", bufs=2)
                # lnc added on PE (identity-lhsT accumulate) for both
                # chunks.  NOTE: once gathers stop pacing the loop, this
                # structure sustains 4.22us/tile (ACT-limited) -- the
                # steady-state pacer is the gather/DMA stream, not PE.
                for c, gr in ((0, gr0), (1, gr1)):
                    for j in range(CHUNK // PIECE):
                        cs = slice(j * PIECE, (j + 1) * PIECE)
                        nc.tensor.matmul(
                            gr[:, cs],
                            lhsT=FT8[:, tok],
                            rhs=FT8[:, c * CHUNK + j * PIECE
                                    : c * CHUNK + (j + 1) * PIECE],
                            start=True, stop=False,
                        )
                    for j in range(CHUNK // PIECE):
                        cs = slice(j * PIECE, (j + 1) * PIECE)
                        nc.tensor.matmul(
                            gr[:, cs], lhsT=idn[:], rhs=ldt[:, c, cs],
                            start=False, stop=True,
                        )
                    nc.scalar.activation(
                        ej[:], gr[:], AF.Exp, accum_out=den2[:, t, c : c + 1]
                    )

            # ---- phase 3: batched epilogue ----
            den = cpool.tile([TILE, NT], f32)
            nc.vector.tensor_tensor(
                out=den[:], in0=den2[:, :, 0], in1=den2[:, :, 1], op=ALU.add,
            )
            ld_all = cpool.tile([TILE, NT], f32)
            nc.scalar.activation(ld_all[:], den[:], AF.Ln)
            nld_all = cpool.tile([TILE, NT], f32)
            nc.vector.tensor_scalar_mul(nld_all[:], ld_all[:], -1.0)
            sh_all = cpool.tile([TILE, NT, P], f32)
            nc.vector.tensor_tensor(
                out=sh_all[:],
                in0=R_all[:],
                in1=nld_all[:].unsqueeze(2).broadcast_to([TILE, NT, P]),
                op=ALU.add,
            )
            tt_all = cpool.tile([TILE, NT * P], f32)
            nc.scalar.activation(
                tt_all[:], sh_all[:].rearrange("p t k -> p (t k)"), AF.Exp,
            )
            spj = cpool.tile([TILE, NT * P], f32)
            cs2 = cpool.tile([TILE, 1], f32)
            nc.scalar.activation(
                spj[:], tt_all[:], AF.Ln, bias=1.0, accum_out=cs2[:],
            )
            ps = psum.tile([TILE, CHUNK], f32, tag="gr0", bufs=1)
            nc.tensor.matmul(
                ps[0:1, 0:1], lhsT=ones[:], rhs=cs2[:], start=True, stop=True
            )
            so = cpool.tile([1, 1], f32)
            nc.vector.tensor_copy(so[:], ps[0:1, 0:1])
            nc.sync.dma_start(out=out[:], in_=so[:])

    nc.compile()
    _CACHE["nc"] = nc
    return nc


def kernel(features, positive_index, negative_index):
    from concourse.bass_utils import run_bass_kernel_spmd

    nc = build_program()

    import ml_dtypes

    e4 = ml_dtypes.float8_e4m3
    feats = np.ascontiguousarray(np.asarray(features, dtype=np.float32))
    pos = np.asarray(positive_index).astype(np.int64)   # [B, N, P]
    neg = np.asarray(negative_index).astype(np.int64)   # [B, N, Q]

    # ln(counts) of all 48 indices, fp8, -16 at zero counts
    allidx = np.concatenate([pos, neg], axis=2)
    lut = np.full(260, -16.0, dtype=np.float32)
    lut[1:] = np.log(np.arange(1, 260, dtype=np.float32))
    base = (np.arange(N, dtype=np.int64) * N)[None, :, None]
    lnc8 = np.empty((B, NT, NCH, 128, CHUNK), dtype=e4)
    for b in range(B):
        cnt = np.bincount((base[0] + allidx[b]).ravel(), minlength=N * N)
        lc = lut[np.minimum(cnt, 259)].reshape(N, N)
        lnc8[b] = (
            lc.reshape(NT, 128, NCH, CHUNK).transpose(0, 2, 1, 3).astype(e4)
        )

    # per (tile, call): k-major flat order (i = k*128 + n -> partition n,
    # slot k), wrapped 16-way and replicated across the 8 partition groups.
    # Slots sorted ascending per token: the loss sums over p, so slot order
    # is free; sorting makes each k-column an order statistic -> gather
    # descriptors cluster in a narrow band of the table (HBM row locality).
    idx16 = np.sort(pos, axis=2).astype(np.int16)
    idx_t = idx16.reshape(B, NT, TILE, NCALL, KC)      # [b, t, n, c, kc]
    flat = idx_t.transpose(0, 1, 3, 4, 2).reshape(B, NT, NCALL, KC * TILE)
    wrapped = flat.reshape(B, NT, NCALL, KC * TILE // 16, 16).transpose(
        0, 1, 2, 4, 3
    )                                                  # [b, t, c, 16, s]
    idxw = np.ascontiguousarray(
        np.tile(wrapped, (1, 1, 1, 8, 1)).astype(np.int16)
    )                                                  # [B, NT, NCALL, 128, 64]

    i8 = np.eye(128, dtype=e4)

    core_ids = list(range(B))
    in_maps = [
        {"features": feats[b], "idxw": idxw[b], "lnc8": lnc8[b], "i8": i8}
        for b in range(B)
    ]

    import os

    trace = bool(int(os.environ.get("BCL_TRACE", "0")))
    res = run_bass_kernel_spmd(nc, in_maps, core_ids, trace=trace)
    _CACHE["last_run"] = res

    s = np.array([res.results[b]["out"][0, 0] for b in range(B)], dtype=np.float64)
    loss = (-s / (P * N)).mean()
    return np.float32(loss)


# revision 29
# speedup vs baseline: 1.0337x; 1.0337x over previous
"""BinaryContrastiveLoss Trainium2 kernel (v4, 224-231us; v3 baseline 264.9us).

Contract: kernel(**inputs) takes the FULL unsharded inputs
  features:       [8, 4096, 128] float32
  positive_index: [8, 4096, 16]  int64
  negative_index: [8, 4096, 32]  int64
and returns the scalar loss (np.float32), matching reference().

Sharding: data-parallel over the batch dim B=8 -> 8 NeuronCores, one
batch element per core.  All gathers are local to a batch element.
Each core computes S_b = sum_n sum_p log1p(exp(pos_dot - ln(denom_n)));
host combines: loss = mean_b( -S_b / (P*N) ).

Device algorithm per core (N=4096 tokens, D=128, P=16, Q=32):
  phase 1: load features in one DMA, batched L2-normalize on DVE
           (sqrt on ACT first so the exp/ln table set loads once),
           bf16 normalized copy -> DRAM gather table; xbar
           dma_start_transpose -> FTbf [d, N] bf16; DVE cast -> FT8 fp8.
  phase 2 per 128-token tile:
    - numerator: 2x 1024-idx dma_gather (bf16 rows, slot-sorted on
      host for HBM locality), DVE multiply + halving tree.
    - denominator: fp8 Gram in two 2048-col f32 PSUM chunks (8x 512-col
      matmuls); ln(count of all 48 indices, fp8, -16 at zero) added via
      identity-lhsT matmuls accumulating into PSUM (PE, both chunks —
      keeps DVE off the slow 1x PSUM path); ACT exp+accum per chunk.
  phase 3: batched epilogue: one TT shift, one exp, one log1p+accum,
           PE ones-matmul partition reduction, scalar out.
"""

import sys

if "/opt/trn_rl_repo" not in sys.path:
    sys.path.insert(0, "/opt/trn_rl_repo")

import numpy as np

B, N, D, P, Q = 8, 4096, 128, 16, 32
TILE = 128
NT = N // TILE
CHUNK = 2048
NCH = N // CHUNK          # denominator chunks per tile
PIECE = 512               # psum bank width (f32)
KC = 8                    # k's per dma_gather call (1024-idx ring limit)
NCALL = P // KC           # gather calls per tile

_CACHE = {}


def build_program():
    if "nc" in _CACHE:
        return _CACHE["nc"]

    from concourse import bacc, bass, mybir, tile

    f32 = mybir.dt.float32
    bf16 = mybir.dt.bfloat16
    fp8 = mybir.dt.float8e4
    i16 = mybir.dt.int16
    AF = mybir.ActivationFunctionType
    ALU = mybir.AluOpType

    nc = bacc.Bacc(None, target_bir_lowering=False, num_swdge_queues=4)
    feats = nc.dram_tensor("features", [N, D], f32, kind="ExternalInput")
    # positive gather indices: per (tile, call) 1024 idx, k-major flat
    # order, wrapped 16-way and replicated across the 8 partition groups
    idxw = nc.dram_tensor(
        "idxw", [NT, NCALL, 128, KC * TILE // 16], i16, kind="ExternalInput"
    )
    # ln(counts of all 48 indices) per token row, fp8, -16 at zero counts
    lnc8 = nc.dram_tensor(
        "lnc8", [NT, NCH, 128, CHUNK], fp8, kind="ExternalInput"
    )
    i8 = nc.dram_tensor("i8", [128, 128], fp8, kind="ExternalInput")
    out = nc.dram_tensor("out", [1, 1], f32, kind="ExternalOutput")
    table = nc.dram_tensor("table", [N, D], bf16)       # gather source

    with tile.TileContext(nc) as tc:
        with (
            tc.tile_pool(name="const", bufs=1) as cpool,
            tc.tile_pool(name="work", bufs=3) as work,
            tc.tile_pool(name="lnc", bufs=6) as lpool,
            tc.tile_pool(name="gather", bufs=10) as gpool,
            tc.tile_pool(name="psum", bufs=2, space="PSUM") as psum,
        ):
            idn = cpool.tile([128, 128], fp8)
            idx_all = cpool.tile([128, NT, NCALL, KC * TILE // 16], i16)
            R_all = cpool.tile([TILE, NT, P], f32)
            den2 = cpool.tile([TILE, NT, NCH], f32)
            ones = cpool.tile([TILE, 1], f32)
            nc.vector.memset(ones[:], 1.0)

            # ---- phase 1: load + normalize + table + transposes ----
            ft_all = cpool.tile([TILE, NT, D], f32)
            nc.sync.dma_start(
                out=ft_all[:], in_=feats[:].rearrange("(t p) d -> p t d", p=TILE)
            )
            nc.sync.dma_start(out=idn[:], in_=i8[:])
            nc.sync.dma_start(
                out=idx_all[:], in_=idxw[:].rearrange("t c p s -> p t c s")
            )
            sqc = cpool.tile([TILE, NT, D], f32)
            ss_all = cpool.tile([TILE, NT], f32)
            rs_all = cpool.tile([TILE, NT], f32)
            ri_all = cpool.tile([TILE, NT], f32)
            nc.vector.tensor_tensor(
                out=sqc[:], in0=ft_all[:], in1=ft_all[:], op=ALU.mult,
            )
            nc.vector.tensor_reduce(
                out=ss_all[:], in_=sqc[:], axis=mybir.AxisListType.X, op=ALU.add,
            )
            nc.vector.reciprocal(rs_all[:], ss_all[:])
            nc.scalar.sqrt(ri_all[:], rs_all[:])
            # batched normalize straight to bf16 (broadcast middle scale)
            fnbf = cpool.tile([TILE, NT, D], bf16)
            nc.vector.tensor_tensor(
                out=fnbf[:],
                in0=ft_all[:],
                in1=ri_all[:].unsqueeze(2).broadcast_to([TILE, NT, D]),
                op=ALU.mult,
            )
            nc.sync.dma_start(
                out=table[:].rearrange("(t p) d -> p t d", p=TILE), in_=fnbf[:]
            )
            # transposed copies: FTbf [d, token] bf16 via xbar, FT8 fp8 cast
            FTbf = cpool.tile([128, N], bf16)
            nc.sync.dma_start_transpose(out=FTbf[:], in_=table[:])
            FT8 = cpool.tile([128, N], fp8)
            nc.vector.tensor_copy(FT8[:], FTbf[:])

            # ---- phase 2: per-tile numerator + denominator ----
            for t in range(NT):
                tok = slice(t * TILE, (t + 1) * TILE)

                # numerator: 1024-idx gather calls + bf16 dot tree on DVE
                g = gpool.tile([TILE, P, D], bf16, tag="g")
                for gc in range(NCALL):
                    nc.gpsimd.dma_gather(
                        out_ap=g[:, gc * KC : (gc + 1) * KC, :],
                        in_ap=table[:],
                        idxs_ap=idx_all[:, t, gc, :],
                        num_idxs=KC * TILE,
                        num_idxs_reg=KC * TILE,
                        elem_size=D,
                        queue_num=(t * NCALL + gc) % 4,
                        single_packet=False,
                    )
                prod = work.tile([TILE, P, D], bf16, tag="prod", bufs=2)
                h1 = work.tile([TILE, P, D // 2], bf16, tag="h1", bufs=2)
                h2 = work.tile([TILE, P, D // 4], bf16, tag="h2", bufs=2)
                nc.vector.tensor_tensor(
                    out=prod[:],
                    in0=g[:],
                    in1=fnbf[:, t, :].unsqueeze(1).broadcast_to([TILE, P, D]),
                    op=ALU.mult,
                )
                nc.vector.tensor_tensor(
                    out=h1[:], in0=prod[:, :, 0 : D // 2],
                    in1=prod[:, :, D // 2 : D], op=ALU.add,
                )
                nc.vector.tensor_tensor(
                    out=h2[:], in0=h1[:, :, 0 : D // 4],
                    in1=h1[:, :, D // 4 : D // 2], op=ALU.add,
                )
                nc.vector.tensor_reduce(
                    out=R_all[:, t, :], in_=h2[:],
                    axis=mybir.AxisListType.X, op=ALU.add,
                )

                # denominator: Gram + lnc (both on PE), exp+accum on ACT
                ldt = lpool.tile([128, NCH, CHUNK], fp8, tag="ldt")
                nc.sync.dma_start(out=ldt[:], in_=lnc8[t].rearrange("c p n -> p c n"))
                gr0 = psum.tile([TILE, CHUNK], f32, tag="gr0", bufs=1)
                gr1 = psum.tile([TILE, CHUNK], f32, tag="gr1", bufs=1)
                ej = work.tile([TILE, CHUNK], bf16, tag="ej", bufs=2)
                # lnc added on PE (identity-lhsT accumulate) for both
                # chunks.  NOTE: once gathers stop pacing the loop, this
                # structure sustains 4.22us/tile (ACT-limited) -- the
                # steady-state pacer is the gather/DMA stream, not PE.
                for c, gr in ((0, gr0), (1, gr1)):
                    for j in range(CHUNK // PIECE):
                        cs = slice(j * PIECE, (j + 1) * PIECE)
                        nc.tensor.matmul(
                            gr[:, cs],
                            lhsT=FT8[:, tok],
                            rhs=FT8[:, c * CHUNK + j * PIECE
                                    : c * CHUNK + (j + 1) * PIECE],
                            start=True, stop=False,
                        )
                    for j in range(CHUNK // PIECE):
                        cs = slice(j * PIECE, (j + 1) * PIECE)
                        nc.tensor.matmul(
                            gr[:, cs], lhsT=idn[:], rhs=ldt[:, c, cs],
                            start=False, stop=True,
                        )
                    nc.scalar.activation(
                        ej[:], gr[:], AF.Exp, accum_out=den2[:, t, c : c + 1]
                    )

            # ---- phase 3: batched epilogue ----
            den = cpool.tile([TILE, NT], f32)
            nc.vector.tensor_tensor(
                out=den[:], in0=den2[:, :, 0], in1=den2[:, :, 1], op=ALU.add,
            )
            ld_all = cpool.tile([TILE, NT], f32)
            nc.scalar.activation(ld_all[:], den[:], AF.Ln)
            nld_all = cpool.tile([TILE, NT], f32)
            nc.vector.tensor_scalar_mul(nld_all[:], ld_all[:], -1.0)
            sh_all = cpool.tile([TILE, NT, P], f32)
            nc.vector.tensor_tensor(
                out=sh_all[:],
                in0=R_all[:],
                in1=nld_all[:].unsqueeze(2).broadcast_to([TILE, NT, P]),
                op=ALU.add,
            )
            tt_all = cpool.tile([TILE, NT * P], f32)
            nc.scalar.activation(
                tt_all[:], sh_all[:].rearrange("p t k -> p (t k)"), AF.Exp,
            )
            spj = cpool.tile([TILE, NT * P], f32)
            cs2 = cpool.tile([TILE, 1], f32)
            nc.scalar.activation(
                spj[:], tt_all[:], AF.Ln, bias=1.0, accum_out=cs2[:],
            )
            ps = psum.tile([TILE, CHUNK], f32, tag="gr0", bufs=1)
            nc.tensor.matmul(
                ps[0:1, 0:1], lhsT=ones[:], rhs=cs2[:], start=True, stop=True
            )
            so = cpool.tile([1, 1], f32)
            nc.vector.tensor_copy(so[:], ps[0:1, 0:1])
            nc.sync.dma_start(out=out[:], in_=so[:])

    nc.compile()
    _CACHE["nc"] = nc
    return nc


def kernel(features, positive_index, negative_index):
    from concourse.bass_utils import run_bass_kernel_spmd

    nc = build_program()

    import ml_dtypes

    e4 = ml_dtypes.float8_e4m3
    feats = np.ascontiguousarray(np.asarray(features, dtype=np.float32))
    pos = np.asarray(positive_index).astype(np.int64)   # [B, N, P]
    neg = np.asarray(negative_index).astype(np.int64)   # [B, N, Q]

    # ln(counts) of all 48 indices, fp8, -16 at zero counts
    allidx = np.concatenate([pos, neg], axis=2)
    lut = np.full(260, -16.0, dtype=np.float32)
    lut[1:] = np.log(np.arange(1, 260, dtype=np.float32))
    base = (np.arange(N, dtype=np.int64) * N)[None, :, None]
    lnc8 = np.empty((B, NT, NCH, 128, CHUNK), dtype=e4)
    for b in range(B):
        cnt = np.bincount((base[0] + allidx[b]).ravel(), minlength=N * N)
        lc = lut[np.minimum(cnt, 259)].reshape(N, N)
        lnc8[b] = (
            lc.reshape(NT, 128, NCH, CHUNK).transpose(0, 2, 1, 3).astype(e4)
        )

    # per (tile, call): k-major flat order (i = k*128 + n -> partition n,
    # slot k), wrapped 16-way and replicated across the 8 partition groups.
    # Slots sorted ascending per token: the loss sums over p, so slot order
    # is free; sorting makes each k-column an order statistic -> gather
    # descriptors cluster in a narrow band of the table (HBM row locality).
    idx16 = np.sort(pos, axis=2).astype(np.int16)
    idx_t = idx16.reshape(B, NT, TILE, NCALL, KC)      # [b, t, n, c, kc]
    flat = idx_t.transpose(0, 1, 3, 4, 2).reshape(B, NT, NCALL, KC * TILE)
    wrapped = flat.reshape(B, NT, NCALL, KC * TILE // 16, 16).transpose(
        0, 1, 2, 4, 3
    )                                                  # [b, t, c, 16, s]
    idxw = np.ascontiguousarray(
        np.tile(wrapped, (1, 1, 1, 8, 1)).astype(np.int16)
    )                                                  # [B, NT, NCALL, 128, 64]

    i8 = np.eye(128, dtype=e4)

    core_ids = list(range(B))
    in_maps = [
        {"features": feats[b], "idxw": idxw[b], "lnc8": lnc8[b], "i8": i8}
        for b in range(B)
    ]

    import os

    trace = bool(int(os.environ.get("BCL_TRACE", "0")))
    res = run_bass_kernel_spmd(nc, in_maps, core_ids, trace=trace)
    _CACHE["last_run"] = res

    s = np.array([res.results[b]["out"][0, 0] for b in range(B)], dtype=np.float64)
    loss = (-s / (P * N)).mean()
    return np.float32(loss)
